# revision 12
# baseline (speedup 1.0000x reference)
"""Trainium2 Bass kernel for y = inputs @ weights.T + bias.

Shapes: inputs [8192, 4096] f32, weights [4096, 4096] f32, bias [4096] f32,
output [8192, 4096] f32.

Strategy:
- Data-parallel across 8 NeuronCores: each core computes 1024 rows of the
  output; weights/bias are replicated.
- Host pre-transposes inputs and weights to K-major layout and converts them
  to bf16 (rel err ~2e-3 at K=4096, tolerance is 2e-2). bf16 halves DMA
  traffic/SBUF footprint vs fp32r and enables Fast Weight Load on the PE
  (LDWEIGHTS ~2x faster), while matmul throughput is the same 1 col/cycle.
- Per core: cache the x-slice KxM [4096,1024] bf16 in SBUF (8.4 MB), stream
  W in [128,512] bf16 tiles, 8 PSUM banks accumulate fp32 over K, bias added
  on DVE during PSUM eviction, y written back as fp32.
- DMA queues are dedicated per stream (w: sync+scalar, x: gpsimd+vector,
  y: gpsimd) so the w-tile stream never suffers head-of-line blocking
  behind x/y transfers (which caused a periodic one-matmul stall).
- A short burst of dummy matmuls on a memset tile right after the engine
  preamble warms the PE HAM clock gate (K=4/8 -> 8/8) while the first real
  tiles are still in flight, so real matmuls run at 2.4 GHz from the start.
- The last n-block runs mb-major on a prefetched w-set so 7 of its 8 PSUM
  evictions/output DMAs overlap the matmul stream; only the final [128,512]
  tile drains after the last matmul (split in halves to pipeline with DMA).
"""

import numpy as np
import ml_dtypes

import concourse.bacc as bacc
import concourse.mybir as mybir
import concourse.tile as tile
from concourse.bass_utils import run_bass_kernel_spmd

N_CORES = 8
N_FULL = 8192  # input rows
K_DIM = 4096  # contraction (in features)
O_DIM = 4096  # out features
M = N_FULL // N_CORES  # rows per core (1024)
P = 128
KO = K_DIM // P  # 32 k-tiles
N_TILE = 512  # moving free dim per matmul (1 PSUM bank of fp32)
N_BLOCKS = O_DIM // N_TILE  # 8
M_BLOCKS = M // P  # 8
N_WARM = 17  # dummy warm-up matmuls (~3.7us at the cold 1.2 GHz clock)

_nc_cache = None


def _build():
    nc = bacc.Bacc(target_bir_lowering=False)

    xT = nc.dram_tensor("xT", [K_DIM, M], mybir.dt.bfloat16, kind="ExternalInput")
    wT = nc.dram_tensor("wT", [K_DIM, O_DIM], mybir.dt.bfloat16, kind="ExternalInput")
    biasr = nc.dram_tensor("biasr", [P, O_DIM], mybir.dt.float32, kind="ExternalInput")
    y = nc.dram_tensor("y", [M, O_DIM], mybir.dt.float32, kind="ExternalOutput")

    xT3 = xT.ap().rearrange("(ko p) m -> p ko m", p=P)
    wT3 = wT.ap().rearrange("(ko p) n -> p ko n", p=P)
    y3 = y.ap().rearrange("(mb p) n -> p mb n", p=P)

    with tile.TileContext(nc) as tc:
        with (
            tc.tile_pool(name="persist", bufs=1) as persist,
            tc.tile_pool(name="wpool", bufs=10) as wpool,
            tc.tile_pool(name="opool", bufs=8) as opool,
            tc.tile_pool(name="ohpool", bufs=4) as ohpool,
            tc.tile_pool(name="psum", bufs=1, space="PSUM") as psum_pool,
        ):
            psums = [
                psum_pool.tile(
                    [P, N_TILE], mybir.dt.float32, tag=f"ps{m}", name=f"ps{m}"
                )
                for m in range(M_BLOCKS)
            ]

            # PE warm-up: dummy matmuls on a memset tile, no DMA deps. They
            # run during the DMA latency of the first real tiles and flip the
            # HAM clock gate to full speed before real matmuls start.
            warm = persist.tile([P, 384], mybir.dt.bfloat16, tag="warm")
            nc.gpsimd.memset(warm[:], 0)
            for _ in range(N_WARM):
                nc.tensor.matmul(
                    psums[M_BLOCKS - 1][:, :256],
                    warm[:, :P],
                    warm[:, P:],
                    start=True,
                    stop=True,
                )

            # x cached in SBUF, one tile per k-slab so matmuls can start as
            # soon as their slab has landed. Only gpsimd/sync/scalar can
            # issue DMAs; sync+scalar are dedicated to the w stream so it
            # never queues behind x/y transfers. x rides gpsimd alone,
            # except slab 0's odd chunks which go out on scalar before any
            # w traffic exists there, halving the first matmul's wait.
            # The first slabs' odd chunks ride scalar ahead of its first w
            # tile (still leaving it ~1us of margin) so the matmul stream
            # never races the x load; everything else rides gpsimd.
            x_sb = []
            bias_sb = [None] * N_BLOCKS
            x_chunks = {0: 4, 1: 2}  # ko -> number of load chunks
            x_eng = {(0, 1): nc.scalar, (0, 3): nc.scalar, (1, 1): nc.scalar}
            for ko in range(KO):
                x_t = persist.tile([P, M], mybir.dt.bfloat16, tag=f"x{ko}")
                nchunk = x_chunks.get(ko, 1)
                csz = M // nchunk
                for c in range(nchunk):
                    xeng = x_eng.get((ko, c), nc.gpsimd)
                    xeng.dma_start(
                        x_t[:, c * csz : (c + 1) * csz],
                        xT3[:, ko, c * csz : (c + 1) * csz],
                    )
                x_sb.append(x_t)

            # bias, then the whole w-set of the last n-block, prefetched on
            # gpsimd (idle once x is cached) so nb=7 can run mb-major with
            # zero DMA dependencies.
            for nb in range(N_BLOCKS):
                b_t = persist.tile([P, N_TILE], mybir.dt.float32, tag=f"bias{nb}")
                nc.gpsimd.dma_start(
                    b_t[:], biasr.ap()[:, nb * N_TILE : (nb + 1) * N_TILE]
                )
                bias_sb[nb] = b_t

            LAST = N_BLOCKS - 1
            w_last = []
            for ko in range(KO):
                w_t = persist.tile([P, N_TILE], mybir.dt.bfloat16, tag=f"wl{ko}")
                nc.gpsimd.dma_start(
                    w_t[:], wT3[:, ko, LAST * N_TILE : (LAST + 1) * N_TILE]
                )
                w_last.append(w_t)

            # n-blocks 0..6: ko-major, streaming w tiles on sync/scalar.
            for nb in range(N_BLOCKS - 1):
                for ko in range(KO):
                    w_t = wpool.tile([P, N_TILE], mybir.dt.bfloat16, tag="w")
                    weng = nc.sync if ko % 2 == 0 else nc.scalar
                    if nb == 0 and ko == 0:
                        # halve the very first tile's DMA latency
                        h = N_TILE // 2
                        weng.dma_start(w_t[:, :h], wT3[:, ko, :h])
                        weng.dma_start(w_t[:, h:], wT3[:, ko, h : N_TILE])
                    else:
                        weng.dma_start(
                            w_t[:], wT3[:, ko, nb * N_TILE : (nb + 1) * N_TILE]
                        )
                    for mb in range(M_BLOCKS):
                        nc.tensor.matmul(
                            psums[mb][:],
                            x_sb[ko][:, mb * P : (mb + 1) * P],
                            w_t[:],
                            start=(ko == 0),
                            stop=(ko == KO - 1),
                        )
                for mb in range(M_BLOCKS):
                    o_t = opool.tile([P, N_TILE], mybir.dt.float32, tag="o")
                    nc.vector.tensor_add(o_t[:], psums[mb][:], bias_sb[nb][:])
                    nc.gpsimd.dma_start(
                        y3[:, mb, nb * N_TILE : (nb + 1) * N_TILE], o_t[:]
                    )

            # Last n-block: mb-major over the prefetched w-set. Each psum
            # bank finishes 32 matmuls before the next starts, so its
            # eviction + y DMA overlap the remaining matmuls; only mb=7
            # drains after the final matmul, in halves to pipeline with DMA.
            H = N_TILE // 2
            for mb in range(M_BLOCKS):
                for ko in range(KO):
                    nc.tensor.matmul(
                        psums[mb][:],
                        x_sb[ko][:, mb * P : (mb + 1) * P],
                        w_last[ko][:],
                        start=(ko == 0),
                        stop=(ko == KO - 1),
                    )
                # finer splits for the very last bank so its eviction
                # pipelines with the output DMA
                nsplit = 4 if mb == M_BLOCKS - 1 else 2
                hs = N_TILE // nsplit
                for h in range(nsplit):
                    o_t = ohpool.tile([P, hs], mybir.dt.float32, tag=f"oh{nsplit}")
                    nc.vector.tensor_add(
                        o_t[:],
                        psums[mb][:, h * hs : (h + 1) * hs],
                        bias_sb[LAST][:, h * hs : (h + 1) * hs],
                    )
                    oeng = nc.sync if h % 2 == 0 else nc.scalar
                    oeng.dma_start(
                        y3[
                            :,
                            mb,
                            LAST * N_TILE + h * hs : LAST * N_TILE + (h + 1) * hs,
                        ],
                        o_t[:],
                    )

    nc.compile()
    return nc


def _get_nc():
    global _nc_cache
    if _nc_cache is None:
        _nc_cache = _build()
    return _nc_cache


def _make_in_maps(inputs, weights, bias):
    x = np.asarray(inputs, dtype=np.float32)
    w = np.asarray(weights, dtype=np.float32)
    b = np.asarray(bias, dtype=np.float32)

    xbf = np.ascontiguousarray(x).astype(ml_dtypes.bfloat16)
    wbf = np.ascontiguousarray(w).astype(ml_dtypes.bfloat16)
    xT = xbf.T  # [K, N_FULL] view
    wT = np.ascontiguousarray(wbf.T)  # [K, O]
    br = np.ascontiguousarray(np.broadcast_to(b[None, :], (P, O_DIM)))

    in_maps = []
    for c in range(N_CORES):
        xTc = np.ascontiguousarray(xT[:, c * M : (c + 1) * M])
        in_maps.append({"xT": xTc, "wT": wT, "biasr": br})
    return in_maps


def kernel(**inputs):
    nc = _get_nc()
    in_maps = _make_in_maps(inputs["inputs"], inputs["weights"], inputs["bias"])
    res = run_bass_kernel_spmd(nc, in_maps, core_ids=list(range(N_CORES)))
    return np.concatenate([r["y"] for r in res.results], axis=0)


def run_traced(inputs, weights, bias, **trace_kwargs):
    """Used by test.py: same computation, returns (output, BassKernelResults)."""
    nc = _get_nc()
    in_maps = _make_in_maps(inputs, weights, bias)
    res = run_bass_kernel_spmd(
        nc, in_maps, core_ids=list(range(N_CORES)), trace=True, **trace_kwargs
    )
    out = np.concatenate([r["y"] for r in res.results], axis=0)
    return out, res


# revision 15
# speedup vs baseline: 1.0076x; 1.0076x over previous
"""Trainium2 Bass kernel for y = inputs @ weights.T + bias.

Shapes: inputs [8192, 4096] f32, weights [4096, 4096] f32, bias [4096] f32,
output [8192, 4096] f32.

Strategy:
- Data-parallel across 8 NeuronCores: each core computes 1024 rows of the
  output; weights/bias are replicated.
- Host pre-transposes inputs and weights to K-major layout and converts them
  to bf16 (rel err ~2e-3 at K=4096, tolerance is 2e-2). bf16 halves DMA
  traffic/SBUF footprint vs fp32r and enables Fast Weight Load on the PE
  (LDWEIGHTS ~2x faster), while matmul throughput is the same 1 col/cycle.
- Per core: cache the x-slice KxM [4096,1024] bf16 in SBUF (8.4 MB), stream
  W in [128,512] bf16 tiles, 8 PSUM banks accumulate fp32 over K, bias added
  on DVE during PSUM eviction, y written back as fp32.
- DMA queues are dedicated per stream (w: sync+scalar, x: gpsimd+vector,
  y: gpsimd) so the w-tile stream never suffers head-of-line blocking
  behind x/y transfers (which caused a periodic one-matmul stall).
- A short burst of dummy matmuls on a memset tile right after the engine
  preamble warms the PE HAM clock gate (K=4/8 -> 8/8) while the first real
  tiles are still in flight, so real matmuls run at 2.4 GHz from the start.
- The last n-block runs mb-major on a prefetched w-set so 7 of its 8 PSUM
  evictions/output DMAs overlap the matmul stream; only the final [128,512]
  tile drains after the last matmul (split in halves to pipeline with DMA).
"""

import numpy as np
import ml_dtypes

import concourse.bacc as bacc
import concourse.mybir as mybir
import concourse.tile as tile
from concourse.bass_utils import run_bass_kernel_spmd

N_CORES = 8
N_FULL = 8192  # input rows
K_DIM = 4096  # contraction (in features)
O_DIM = 4096  # out features
M = N_FULL // N_CORES  # rows per core (1024)
P = 128
KO = K_DIM // P  # 32 k-tiles
N_TILE = 512  # moving free dim per matmul (1 PSUM bank of fp32)
N_BLOCKS = O_DIM // N_TILE  # 8
M_BLOCKS = M // P  # 8
N_WARM = 17  # dummy warm-up matmuls (~3.7us at the cold 1.2 GHz clock)

_nc_cache = None


def _build():
    nc = bacc.Bacc(target_bir_lowering=False)

    xT = nc.dram_tensor("xT", [K_DIM, M], mybir.dt.bfloat16, kind="ExternalInput")
    wT = nc.dram_tensor("wT", [K_DIM, O_DIM], mybir.dt.bfloat16, kind="ExternalInput")
    biasr = nc.dram_tensor("biasr", [P, O_DIM], mybir.dt.float32, kind="ExternalInput")
    y = nc.dram_tensor("y", [M, O_DIM], mybir.dt.float32, kind="ExternalOutput")

    xT3 = xT.ap().rearrange("(ko p) m -> p ko m", p=P)
    wT3 = wT.ap().rearrange("(ko p) n -> p ko n", p=P)
    y3 = y.ap().rearrange("(mb p) n -> p mb n", p=P)

    with tile.TileContext(nc) as tc:
        with (
            tc.tile_pool(name="persist", bufs=1) as persist,
            tc.tile_pool(name="wpool", bufs=10) as wpool,
            tc.tile_pool(name="opool", bufs=8) as opool,
            tc.tile_pool(name="ohpool", bufs=4) as ohpool,
            tc.tile_pool(name="psum", bufs=1, space="PSUM") as psum_pool,
        ):
            psums = [
                psum_pool.tile(
                    [P, N_TILE], mybir.dt.float32, tag=f"ps{m}", name=f"ps{m}"
                )
                for m in range(M_BLOCKS)
            ]

            # PE warm-up: dummy matmuls on a memset tile, no DMA deps. They
            # run during the DMA latency of the first real tiles and flip the
            # HAM clock gate to full speed before real matmuls start.
            warm = persist.tile([P, 384], mybir.dt.bfloat16, tag="warm")
            nc.gpsimd.memset(warm[:], 0)
            for _ in range(N_WARM):
                nc.tensor.matmul(
                    psums[M_BLOCKS - 1][:, :256],
                    warm[:, :P],
                    warm[:, P:],
                    start=True,
                    stop=True,
                )

            # x cached in SBUF, one tile per k-slab so matmuls can start as
            # soon as their slab has landed. Only gpsimd/sync/scalar can
            # issue DMAs; sync+scalar are dedicated to the w stream so it
            # never queues behind x/y transfers. x rides gpsimd alone,
            # except slab 0's odd chunks which go out on scalar before any
            # w traffic exists there, halving the first matmul's wait.
            # The first four slabs arrive in fine-grained chunks split over
            # gpsimd+scalar (scalar's first w tile was moved to sync, so its
            # early window is free) - a late early slab would idle the PE
            # past the HAM window and trigger an expensive re-throttle.
            x_sb = []
            bias_sb = [None] * N_BLOCKS
            x_chunks = {0: 4, 1: 2, 2: 2, 3: 2}  # ko -> number of load chunks
            for ko in range(KO):
                x_t = persist.tile([P, M], mybir.dt.bfloat16, tag=f"x{ko}")
                nchunk = x_chunks.get(ko, 1)
                csz = M // nchunk
                for c in range(nchunk):
                    xeng = nc.scalar if c % 2 == 1 else nc.gpsimd
                    xeng.dma_start(
                        x_t[:, c * csz : (c + 1) * csz],
                        xT3[:, ko, c * csz : (c + 1) * csz],
                    )
                x_sb.append(x_t)

            # bias, then the whole w-set of the last n-block, prefetched on
            # gpsimd (idle once x is cached) so nb=7 can run mb-major with
            # zero DMA dependencies.
            for nb in range(N_BLOCKS):
                b_t = persist.tile([P, N_TILE], mybir.dt.float32, tag=f"bias{nb}")
                nc.gpsimd.dma_start(
                    b_t[:], biasr.ap()[:, nb * N_TILE : (nb + 1) * N_TILE]
                )
                bias_sb[nb] = b_t

            LAST = N_BLOCKS - 1
            w_last = []
            for ko in range(KO):
                w_t = persist.tile([P, N_TILE], mybir.dt.bfloat16, tag=f"wl{ko}")
                nc.gpsimd.dma_start(
                    w_t[:], wT3[:, ko, LAST * N_TILE : (LAST + 1) * N_TILE]
                )
                w_last.append(w_t)

            # n-blocks 0..6: ko-major, streaming w tiles on sync/scalar.
            for nb in range(N_BLOCKS - 1):
                for ko in range(KO):
                    w_t = wpool.tile([P, N_TILE], mybir.dt.bfloat16, tag="w")
                    # ko parity splits w over sync/scalar; w01 goes to sync
                    # too so scalar's early x chunks don't delay it.
                    weng = nc.sync if (ko % 2 == 0 or (nb == 0 and ko == 1)) else nc.scalar
                    if nb == 0 and ko == 0:
                        # halve the very first tile's DMA latency
                        h = N_TILE // 2
                        weng.dma_start(w_t[:, :h], wT3[:, ko, :h])
                        weng.dma_start(w_t[:, h:], wT3[:, ko, h : N_TILE])
                    else:
                        weng.dma_start(
                            w_t[:], wT3[:, ko, nb * N_TILE : (nb + 1) * N_TILE]
                        )
                    for mb in range(M_BLOCKS):
                        nc.tensor.matmul(
                            psums[mb][:],
                            x_sb[ko][:, mb * P : (mb + 1) * P],
                            w_t[:],
                            start=(ko == 0),
                            stop=(ko == KO - 1),
                        )
                for mb in range(M_BLOCKS):
                    o_t = opool.tile([P, N_TILE], mybir.dt.float32, tag="o")
                    nc.vector.tensor_add(o_t[:], psums[mb][:], bias_sb[nb][:])
                    nc.gpsimd.dma_start(
                        y3[:, mb, nb * N_TILE : (nb + 1) * N_TILE], o_t[:]
                    )

            # Last n-block: mb-major over the prefetched w-set. Each psum
            # bank finishes 32 matmuls before the next starts, so its
            # eviction + y DMA overlap the remaining matmuls; only mb=7
            # drains after the final matmul, in halves to pipeline with DMA.
            H = N_TILE // 2
            for mb in range(M_BLOCKS):
                for ko in range(KO):
                    nc.tensor.matmul(
                        psums[mb][:],
                        x_sb[ko][:, mb * P : (mb + 1) * P],
                        w_last[ko][:],
                        start=(ko == 0),
                        stop=(ko == KO - 1),
                    )
                for h in range(2):
                    o_t = ohpool.tile([P, H], mybir.dt.float32, tag="oh")
                    nc.vector.tensor_add(
                        o_t[:],
                        psums[mb][:, h * H : (h + 1) * H],
                        bias_sb[LAST][:, h * H : (h + 1) * H],
                    )
                    oeng = nc.sync if h % 2 == 0 else nc.scalar
                    oeng.dma_start(
                        y3[
                            :,
                            mb,
                            LAST * N_TILE + h * H : LAST * N_TILE + (h + 1) * H,
                        ],
                        o_t[:],
                    )

    nc.compile()
    return nc


def _get_nc():
    global _nc_cache
    if _nc_cache is None:
        _nc_cache = _build()
    return _nc_cache


def _make_in_maps(inputs, weights, bias):
    x = np.asarray(inputs, dtype=np.float32)
    w = np.asarray(weights, dtype=np.float32)
    b = np.asarray(bias, dtype=np.float32)

    xbf = np.ascontiguousarray(x).astype(ml_dtypes.bfloat16)
    wbf = np.ascontiguousarray(w).astype(ml_dtypes.bfloat16)
    xT = xbf.T  # [K, N_FULL] view
    wT = np.ascontiguousarray(wbf.T)  # [K, O]
    br = np.ascontiguousarray(np.broadcast_to(b[None, :], (P, O_DIM)))

    in_maps = []
    for c in range(N_CORES):
        xTc = np.ascontiguousarray(xT[:, c * M : (c + 1) * M])
        in_maps.append({"xT": xTc, "wT": wT, "biasr": br})
    return in_maps


def kernel(**inputs):
    nc = _get_nc()
    in_maps = _make_in_maps(inputs["inputs"], inputs["weights"], inputs["bias"])
    res = run_bass_kernel_spmd(nc, in_maps, core_ids=list(range(N_CORES)))
    return np.concatenate([r["y"] for r in res.results], axis=0)


def run_traced(inputs, weights, bias, **trace_kwargs):
    """Used by test.py: same computation, returns (output, BassKernelResults)."""
    nc = _get_nc()
    in_maps = _make_in_maps(inputs, weights, bias)
    res = run_bass_kernel_spmd(
        nc, in_maps, core_ids=list(range(N_CORES)), trace=True, **trace_kwargs
    )
    out = np.concatenate([r["y"] for r in res.results], axis=0)
    return out, res
